# revision 1
# baseline (speedup 1.0000x reference)
"""CP-factorized embedding lookup on 8 TRN2 NeuronCores.

Reference computes full[a,b,c,d,e,f] = sum_r U0[a,r]*...*U5[f,r], reshapes to a
(50000, 512) table, and gathers rows by x. We never materialize the table:

  out[n, e] = sum_r (U0[a_n,r]*U1[b_n,r]*U2[c_n,r]) * (U3[d,r]*U4[e2,r]*U5[f,r])
            = sum_r V[n, r] * W[e, r]

with v = 1000a + 25b + c and e = 64d + 8e2 + f.

Per core (1024 indices, data-parallel over the 8192 total), in two pipelined
512-index halves:
  1. broadcast x across 115 partitions (50+40+25 stacked factor rows) and
     decompose it in place with per-partition constants in a short 16-bit
     DVE chain (4x perf mode):
       rows  0:50  -> a      = floor(v/1000)   (1000 when v == 0: see below)
       rows 50:90  -> b + 50 = floor(v/25) - 40*floor(v/1000) + 50
       rows 90:115 -> c + 90 = (v-25000) - 25*(floor(v/25)-1000) + 90
     floor(v/d) = f32->i16 cast of (v + bias)*(1/d); the HW cast rounds to
     nearest even, bias = -(d/2 - 0.5) puts the value mid-interval, so the
     result is exact. Block 2 is offset by -25000 to fit int16. The
     padding mask is folded in: rows 0:50 use s2 = min(v, 1) and
     diff = a - 1000*s2 + 1000, which equals a for v > 0 and 1000 (no
     one-hot hit -> zero row) for v == 0.
  2. one-hot[115, 512] = is_equal(diff, iota); gather via one PE matmul
     with block-diag stacked [U0;U1;U2] as lhsT -> psum[96, 512];
     V = elementwise product of the three 32-row blocks
  3. W[32, 512] = Khatri-Rao of U3,U4,U5 built with two broadcast multiplies
     (U3/U4/U5 transposed on-chip through the PE)
  4. out chunk c: matmul(lhsT=V[:,128j:128j+128], rhs=W) -> psum, two chunks
     batched per [128, 1024] psum pair, one Scalar-engine copy -> SBUF,
     one DMA per 256 output rows

All small constant operands (decomposition table, iota, identity, stacked
U3..U5, block-diagonal [U0;U1;U2]) are packed host-side into one aux input
(pure rearrangement/zero-padding -- all arithmetic stays on device) so the
front end costs a single small DMA. Matmul operands are produced as float32r
(tf32-like, 1 row/cycle vs 4 for float32); one-hot entries are exact in any
dtype and the factor rounding error is ~1e-4 relative, far inside tolerance.
"""

import numpy as np

import concourse.bass as bass
import concourse.mybir as mybir
import concourse.tile as tile
from concourse import bacc
from concourse.bass_utils import run_bass_kernel_spmd

F32 = mybir.dt.float32
F32R = mybir.dt.float32r
I32 = mybir.dt.int32
I16 = mybir.dt.int16
U16 = mybir.dt.uint16
ALU = mybir.AluOpType

N_CORES = 8
PER_CORE = 1024           # indices per core (8192 / 8)
HALF = 512                # pipeline granularity (one PSUM bank of columns)
EMB = 512
RANK = 32
KV = 115                  # 50 + 40 + 25 stacked vocab-factor rows
MV = 96                   # 3 * RANK stacked outputs

R1000 = float(np.float32(1.0 / 1000.0))
R25 = float(np.float32(1.0 / 25.0))

# aux layout: [115, 7 + 24 + 32 + 96]
CC_OFF = 0      # [115, 7] decomposition constants + iota
ID_OFF = 7      # [24, 24] identity (rows 0:24)
U345_OFF = 31   # [24, 32] stacked U3;U4;U5 (rows 0:24)
UBLK_OFF = 63   # [115, 96] block-diag [U0;U1;U2]
ONES_OFF = 159  # [1, 115] row of ones (lhsT of the broadcast matmul)
AUX_W = 274

# matmul operand dtype: float32r streams 1 row/cycle (vs 4 for float32).
MM_DT = F32R


def _const_table() -> np.ndarray:
    """[115, 7] per-partition constants: b1, R1, b2, R2, K, OFF, iota.

    Chain (s1, s2 are f32->i16 floor stages; the cast rounds to nearest):
      s1 = rint((v + b1) * R1);  s2 = rint((v + b2) * R2)
      (rows 0:50 overwrite: s2 = min(v, 1))
      diff = s1 - (K*s2 - OFF)  ; onehot = (diff == iota)
    """
    cc = np.zeros((KV, 7), np.float32)
    rows = ((0, 50), (50, 90), (90, 115))
    vals = [
        # s1 = a; s2 = min(v,1); hit iff a == 1000*s2 - 1000 + p
        (-499.5, R1000, 0.0, 1.0, 1000.0, 1000.0),
        # s1 = q25; s2 = a; hit iff q25 == 40a - 50 + p  (p abs. row 50..89)
        (-12.0, R25, -499.5, R1000, 40.0, 50.0),
        # s1 = v-25000; s2 = q25-1000; hit iff s1 == 25*s2 - 90 + p
        (-25000.0, 1.0, -25012.0, R25, 25.0, 90.0),
    ]
    for (lo, hi), v6 in zip(rows, vals):
        cc[lo:hi, 0:6] = np.float32(v6)
    # OFF2 = OFF - row: tkp = K*s2 - OFF2 and the one-hot becomes a single
    # fused tensor_tensor is_equal(s1, tkp)
    cc[:, 5] -= np.arange(KV, dtype=np.float32)
    return cc


def _aux_table(us: list[np.ndarray]) -> np.ndarray:
    aux = np.zeros((KV, AUX_W), np.float32)
    aux[:, CC_OFF:CC_OFF + 7] = _const_table()
    aux[0:24, ID_OFF:ID_OFF + 24] = np.eye(24, dtype=np.float32)
    aux[0:8, U345_OFF:U345_OFF + 32] = us[3]
    aux[8:16, U345_OFF:U345_OFF + 32] = us[4]
    aux[16:24, U345_OFF:U345_OFF + 32] = us[5]
    aux[0:50, UBLK_OFF:UBLK_OFF + 32] = us[0]
    aux[50:90, UBLK_OFF + 32:UBLK_OFF + 64] = us[1]
    aux[90:115, UBLK_OFF + 64:UBLK_OFF + 96] = us[2]
    aux[0, ONES_OFF:ONES_OFF + KV] = 1.0
    return aux


def build():
    nc = bacc.Bacc("TRN2", target_bir_lowering=False, debug=False)

    x = nc.dram_tensor("x", [PER_CORE], I32, kind="ExternalInput")
    aux_d = nc.dram_tensor("aux", [KV, AUX_W], F32, kind="ExternalInput")
    out = nc.dram_tensor("out", [PER_CORE, EMB], F32, kind="ExternalOutput")

    NH = PER_CORE // HALF   # 2 halves
    NC2 = HALF // 256       # 2 two-chunk groups per half

    with tile.TileContext(nc) as tc:
        with (
            tc.tile_pool(name="const", bufs=1) as cpool,
            tc.tile_pool(name="work", bufs=2) as wpool,
            tc.tile_pool(name="vpsum", bufs=2, space="PSUM") as ppool,
            tc.tile_pool(name="osb", bufs=2) as opool,
            tc.tile_pool(name="opsum", bufs=2, space="PSUM") as oppool,
        ):
            # ---- broadcast x across the 115 stacked factor rows (one
            # full-width DMA on the sync ring); aux lands in parallel on
            # the scalar ring.
            aux = cpool.tile([KV, AUX_W], F32)
            nc.sync.dma_start(out=aux[:], in_=aux_d[:])
            xrep = cpool.tile([KV, PER_CORE], I32)
            nc.sync.dma_start(
                out=xrep[:], in_=x[:].unsqueeze(0).partition_broadcast(KV)
            )
            cc = aux[:, CC_OFF:CC_OFF + 7]
            idm = aux[0:24, ID_OFF:ID_OFF + 24]
            u345 = aux[0:24, U345_OFF:U345_OFF + 32]

            # f32r-rounded copy of the block-diag factors for the gather mm
            ublk = cpool.tile([KV, MV], MM_DT)
            nc.vector.tensor_copy(out=ublk[:], in_=aux[:, UBLK_OFF:UBLK_OFF + 96])

            # ---- W[r, e] = U3[d,r] * U4[e2,r] * U5[f,r],  e = 64d + 8e2 + f
            u345t_ps = ppool.tile([RANK, 24], F32, tag="pv")
            nc.tensor.transpose(u345t_ps[:], u345, idm)
            u345t = cpool.tile([RANK, 24], F32)
            nc.scalar.copy(out=u345t[:], in_=u345t_ps[:])
            t45 = cpool.tile([RANK, 64], F32)
            nc.vector.tensor_tensor(
                out=t45[:].rearrange("r (e f) -> r e f", e=8),
                in0=u345t[:, 8:16].unsqueeze(2).broadcast_to([RANK, 8, 8]),
                in1=u345t[:, 16:24].unsqueeze(1).broadcast_to([RANK, 8, 8]),
                op=ALU.mult,
            )
            wt = cpool.tile([RANK, EMB], MM_DT)
            nc.vector.tensor_tensor(
                out=wt[:].rearrange("r (d ef) -> r d ef", d=8),
                in0=u345t[:, 0:8].unsqueeze(2).broadcast_to([RANK, 8, 64]),
                in1=t45[:].unsqueeze(1).broadcast_to([RANK, 8, 64]),
                op=ALU.mult,
            )

            # ---- full-width 5-op decomposition chain straight off the
            # int32 broadcast (mixed int-in/f32-scalar tensor_scalar is
            # exact on HW: internal fp32 ALU + round-to-nearest int cast)
            s1 = cpool.tile([KV, PER_CORE], I16)
            nc.vector.tensor_scalar(
                out=s1[:], in0=xrep[:], scalar1=cc[:, 0:1], scalar2=cc[:, 1:2],
                op0=ALU.add, op1=ALU.mult,
            )
            s2 = cpool.tile([KV, PER_CORE], I16)
            nc.vector.tensor_scalar(
                out=s2[:], in0=xrep[:], scalar1=cc[:, 2:3], scalar2=cc[:, 3:4],
                op0=ALU.add, op1=ALU.mult,
            )
            # rows 0:50: s2 = min(v, 1) -> folds the v==0 padding mask into
            # the block-0 one-hot (no hit for v == 0 -> zero output row)
            nc.vector.tensor_scalar(
                out=s2[0:50, :], in0=xrep[0:50, :], scalar1=1.0, scalar2=1.0,
                op0=ALU.min, op1=ALU.mult,
            )
            tkp = cpool.tile([KV, PER_CORE], I16)
            nc.vector.tensor_scalar(
                out=tkp[:], in0=s2[:], scalar1=cc[:, 4:5], scalar2=cc[:, 5:6],
                op0=ALU.mult, op1=ALU.subtract,
            )
            onehot = cpool.tile([KV, PER_CORE], MM_DT)
            nc.vector.tensor_tensor(
                out=onehot[:], in0=s1[:], in1=tkp[:], op=ALU.is_equal
            )

            for h in range(NH):
                pv = ppool.tile([MV, HALF], F32, name=f"pv_{h}", tag="pv")
                nc.tensor.matmul(
                    pv[:], lhsT=ublk[:],
                    rhs=onehot[:, h * HALF:(h + 1) * HALF],
                    start=True, stop=True,
                )
                # DVE may read only one PSUM operand per op: stage block 0
                # to SBUF on the Scalar engine first.
                s0 = wpool.tile([RANK, HALF], F32, name=f"s0_{h}", tag="s0")
                nc.scalar.copy(out=s0[:], in_=pv[0:32, :])
                v01 = wpool.tile([RANK, HALF], F32, name=f"v01_{h}", tag="v01")
                nc.vector.tensor_tensor(
                    out=v01[:], in0=s0[:], in1=pv[32:64, :], op=ALU.mult
                )
                vth = cpool.tile([RANK, HALF], MM_DT, name=f"vt_{h}")
                nc.vector.tensor_tensor(
                    out=vth[:], in0=v01[:], in1=pv[64:96, :], op=ALU.mult
                )

                # two output chunks batched per [128, 1024] psum pair
                for g in range(NC2):
                    po2 = oppool.tile([128, 2 * EMB], F32, name=f"po_{h}{g}",
                                      tag="po")
                    for j in range(2):
                        nc.tensor.matmul(
                            po2[:, j * EMB:(j + 1) * EMB],
                            lhsT=vth[:, (2 * g + j) * 128:(2 * g + j + 1) * 128],
                            rhs=wt[:],
                            start=True, stop=True,
                        )
                    osb = opool.tile([128, 2 * EMB], F32, name=f"osb_{h}{g}",
                                     tag="osb")
                    if g == 0:
                        nc.scalar.copy(out=osb[:], in_=po2[:])
                    else:
                        nc.vector.tensor_copy(out=osb[:], in_=po2[:])
                    row0 = h * HALF + g * 256
                    nc.sync.dma_start(
                        out=out[row0:row0 + 256, :].rearrange(
                            "(j p) e -> p j e", p=128
                        ),
                        in_=osb[:].rearrange("p (j e) -> p j e", j=2),
                    )

    nc.compile()
    return nc


_CACHE: dict = {}


def _get_nc():
    if "nc" not in _CACHE:
        _CACHE["nc"] = build()
    return _CACHE["nc"]


def run(inputs, **spmd_kwargs):
    nc = _get_nc()
    x = np.ascontiguousarray(inputs["x"].reshape(-1), dtype=np.int32)
    us = [
        np.ascontiguousarray(inputs[f"U{j}"], dtype=np.float32) for j in range(6)
    ]
    aux = _aux_table(us)
    in_maps = []
    for i in range(N_CORES):
        in_maps.append({"x": x[i * PER_CORE:(i + 1) * PER_CORE], "aux": aux})
    res = run_bass_kernel_spmd(
        nc, in_maps, core_ids=list(range(N_CORES)), **spmd_kwargs
    )
    shards = [np.asarray(res.results[i]["out"]) for i in range(N_CORES)]
    full = np.concatenate(shards, axis=0).reshape(4, 2048, EMB)
    return full.astype(np.float32, copy=False), res


def kernel(**inputs) -> np.ndarray:
    return run(inputs)[0]



# revision 7
# speedup vs baseline: 1.1820x; 1.1820x over previous
"""CP-factorized embedding lookup on 8 TRN2 NeuronCores (v2).

Reference computes full[a,b,c,d,e,f] = sum_r U0[a,r]*...*U5[f,r], reshapes to a
(50000, 512) table, and gathers rows by x. We never materialize the table:

  out[n, e] = sum_r (U0[a_n,r]*U1[b_n,r]*U2[c_n,r]) * (U3[d,r]*U4[e2,r]*U5[f,r])
            = sum_r V[n, r] * W[e, r]

with v = 1000a + 25b + c and e = 64d + 8e2 + f.

Per core (1024 indices, data-parallel over the 8192 total):
  1. x lands once as [8, 128] (4 KB, no HBM broadcast). Digits are computed on
     tiny tiles with Scalar-ACT affine ops (q=floor(v/25), a=floor(v/1000),
     25q, 40a, t=relu(1000-1000v)) and three DVE tensor_tensor ops that write
     h0=t+a, h1=q-40a, h2=v-25q into one [24, 128] fp16 tile (partition
     j / 8+j / 16+j holds chunk j of h0/h1/h2). floor(v/d) is the
     f32->i16 round-to-nearest trick: rint((v + bias)*(1/d)) with bias putting
     the value mid-interval. The v==0 padding row is folded in: h0 = 1000 for
     v == 0 which hits no block-0 row, so V0 = 0 and the output row is zero.
  2. Eight tiny PE matmuls with 0/1 selector weights B24_j broadcast the three
     h rows across the 115 stacked vocab-factor partitions (psum[115, 1024]),
     then one is_equal tensor_scalar per 512-half against a per-partition iota
     produces the fp16 one-hot.
  3. One PE matmul per half gathers all three factor rows at once with the
     block-diag [U0;U1;U2] as stationary operand -> psum[96, 512]; V is the
     elementwise product of the three 32-row blocks (fp16, 2x DVE mode).
  4. out chunk: matmul(lhsT=V[:, 128-slice], rhs=W[32, 512]) -> psum, two
     chunks batched per [128, 1024] psum pair chosen so partition p holds two
     CONSECUTIVE output rows (8p+j, 8p+j+1); one psum->SBUF fp16 copy and one
     DMA per 256 output rows writes 2 KB contiguous per partition.

The output tensor is fp16 (halves HBM write traffic; rounding is ~2^-11
relative, far inside the 2e-2 tolerance) and is cast back to f32 on host.
All matmul operands are fp16: one-hots/selectors/identities are exact 0/1,
h-values are integers <= 2000 (exact in fp16 up to 2048), and the CP factors
round at ~5e-4 relative. All arithmetic on factor VALUES stays on device;
host-side work is dtype casting and zero-padded packing only.
"""

import numpy as np

import concourse.bass as bass
import concourse.mybir as mybir
import concourse.tile as tile
from concourse import bacc
from concourse.bass_utils import run_bass_kernel_spmd

F32 = mybir.dt.float32
F16 = mybir.dt.float16
I32 = mybir.dt.int32
I16 = mybir.dt.int16
ALU = mybir.AluOpType
ACT = mybir.ActivationFunctionType

N_CORES = 8
PER_CORE = 1024           # indices per core (8192 / 8)
HALF = 512
EMB = 512
RANK = 32
KV = 115                  # 50 + 40 + 25 stacked vocab-factor rows
MV = 96                   # 3 * RANK stacked gather outputs
NJ = 8                    # index chunks of 128 (x laid out [8, 128])

R25 = float(np.float32(1.0 / 25.0))
R1000 = float(np.float32(1.0 / 1000.0))

# auxh (fp16) layout: [115, AUXH_W]
B96_OFF = 0      # rows 0:96,  cols 0:920      8x B96_j [96, 115]
UBLK_OFF = 920   # rows 0:115, cols 920:1016   block-diag [U0;U1;U2]
U345_OFF = 1016  # rows 0:24,  cols 1016:1048  stacked U3;U4;U5
ID24_OFF = 1048  # rows 0:24,  cols 1048:1072  identity 24
IOTA_OFF = 1072  # rows 0:115, col 1072        per-partition local index
AUXH_W = 1074


def _auxh_table(us: list[np.ndarray]) -> np.ndarray:
    aux = np.zeros((KV, AUXH_W), np.float16)
    # B96_j[k, p] = 1 iff k == 32*block(p) + j  (h rows live at partition
    # bases 0/32/64 because engine APs must start at multiples of 32)
    blk = np.zeros(KV, np.int64)
    blk[50:90] = 1
    blk[90:115] = 2
    for j in range(NJ):
        m = np.zeros((96, KV), np.float16)
        m[32 * blk + j, np.arange(KV)] = 1.0
        aux[0:96, B96_OFF + KV * j:B96_OFF + KV * (j + 1)] = m
    aux[0:50, UBLK_OFF:UBLK_OFF + 32] = us[0].astype(np.float16)
    aux[50:90, UBLK_OFF + 32:UBLK_OFF + 64] = us[1].astype(np.float16)
    aux[90:115, UBLK_OFF + 64:UBLK_OFF + 96] = us[2].astype(np.float16)
    aux[0:8, U345_OFF:U345_OFF + 32] = us[3].astype(np.float16)
    aux[8:16, U345_OFF:U345_OFF + 32] = us[4].astype(np.float16)
    aux[16:24, U345_OFF:U345_OFF + 32] = us[5].astype(np.float16)
    aux[0:24, ID24_OFF:ID24_OFF + 24] = np.eye(24, dtype=np.float16)
    iota = np.concatenate([np.arange(50), np.arange(40), np.arange(25)])
    aux[:, IOTA_OFF] = iota.astype(np.float16)
    return aux


def build():
    nc = bacc.Bacc("TRN2", target_bir_lowering=False, debug=False)

    x = nc.dram_tensor("x", [PER_CORE], I32, kind="ExternalInput")
    auxh_d = nc.dram_tensor("auxh", [KV, AUXH_W], F16, kind="ExternalInput")
    out = nc.dram_tensor("out", [PER_CORE, EMB], F16, kind="ExternalOutput")
    outv = out[:].rearrange("(j p) e -> p j e", p=128)  # partition p, row 128j+p

    with tile.TileContext(nc) as tc:
        with (
            tc.tile_pool(name="const", bufs=1) as cpool,
            tc.tile_pool(name="work", bufs=2) as wpool,
            tc.tile_pool(name="pbc", bufs=2, space="PSUM") as bcpool,
            tc.tile_pool(name="pv", bufs=2, space="PSUM") as pvpool,
            tc.tile_pool(name="po", bufs=2, space="PSUM") as popool,
        ):
            # ---- input DMAs
            xt = cpool.tile([NJ, 128], I32)
            nc.sync.dma_start(out=xt[:], in_=x[:].rearrange("(j n) -> j n", j=NJ))
            auxh = cpool.tile([KV, AUXH_W], F16)
            nc.scalar.dma_start(out=auxh[:], in_=auxh_d[:])

            ublk = auxh[:, UBLK_OFF:UBLK_OFF + MV]
            u345 = auxh[0:24, U345_OFF:U345_OFF + 32]
            id24 = auxh[0:24, ID24_OFF:ID24_OFF + 24]
            iota16 = auxh[:, IOTA_OFF:IOTA_OFF + 1]

            # per-partition iota as f32 for the psum compare
            iota = cpool.tile([KV, 1], F32)
            nc.vector.tensor_copy(out=iota[:], in_=iota16)

            # ---- W[r, e] = U3[d,r]*U4[e2,r]*U5[f,r],  e = 64d + 8e2 + f
            u345t_ps = pvpool.tile([RANK, 24], F16, tag="pv")
            nc.tensor.transpose(u345t_ps[:], u345, id24)
            u345t = cpool.tile([RANK, 24], F16)
            nc.scalar.copy(out=u345t[:], in_=u345t_ps[:])
            t45 = cpool.tile([RANK, 64], F16)
            nc.vector.tensor_tensor(
                out=t45[:].rearrange("r (e f) -> r e f", e=8),
                in0=u345t[:, 8:16].unsqueeze(2).broadcast_to([RANK, 8, 8]),
                in1=u345t[:, 16:24].unsqueeze(1).broadcast_to([RANK, 8, 8]),
                op=ALU.mult,
            )
            wt = cpool.tile([RANK, EMB], F16)
            nc.vector.tensor_tensor(
                out=wt[:].rearrange("r (d ef) -> r d ef", d=8),
                in0=u345t[:, 0:8].unsqueeze(2).broadcast_to([RANK, 8, 64]),
                in1=t45[:].unsqueeze(1).broadcast_to([RANK, 8, 64]),
                op=ALU.mult,
            )

            # ---- digit decomposition on [8, 128] tiles.
            # q = floor(v/25), a = floor(v/1000) via mid-interval f32->i16
            # round; 25q/40a exact integer scales; t = relu(1000 - 1000v).
            q = cpool.tile([NJ, 128], I16)
            nc.scalar.activation(q[:], xt[:], ACT.Copy,
                                 bias=float(np.float32(-12.0 * R25)), scale=R25)
            a = cpool.tile([NJ, 128], I16)
            nc.scalar.activation(a[:], xt[:], ACT.Copy,
                                 bias=float(np.float32(-499.5 * R1000)),
                                 scale=R1000)
            q25 = cpool.tile([NJ, 128], I32)
            nc.scalar.activation(q25[:], q[:], ACT.Copy, bias=0.0, scale=25.0)
            a40 = cpool.tile([NJ, 128], I16)
            nc.scalar.activation(a40[:], a[:], ACT.Copy, bias=0.0, scale=40.0)
            b1000 = cpool.tile([NJ, 1], F32)
            nc.gpsimd.memset(b1000[:], 1000.0)
            tt0 = cpool.tile([NJ, 128], I16)
            nc.scalar.activation(tt0[:], xt[:], ACT.Relu, bias=b1000[:, 0:1],
                                 scale=-1000.0)

            # h3T[96, 128]: rows 0:8 = h0 = t + a (1000 for v==0, else a),
            # rows 32:40 = h1 = q - 40a = b, rows 64:72 = h2 = v - 25q = c.
            # Unused rows are zeroed (zero selector weights must not meet NaN
            # garbage: 0 * NaN = NaN).
            h3T = cpool.tile([96, 128], F16)
            nc.gpsimd.memset(h3T[:], 0.0)
            nc.vector.tensor_tensor(out=h3T[0:8, :], in0=tt0[:], in1=a[:],
                                    op=ALU.add)
            nc.vector.tensor_tensor(out=h3T[32:40, :], in0=q[:], in1=a40[:],
                                    op=ALU.subtract)
            nc.vector.tensor_tensor(out=h3T[64:72, :], in0=xt[:], in1=q25[:],
                                    op=ALU.subtract)

            # ---- broadcast h across the 115 factor rows (8 selector matmuls)
            # and compare against the per-partition local index -> one-hot.
            onehot = cpool.tile([KV, PER_CORE], F16)
            pbc = []
            for h in range(2):
                p = bcpool.tile([KV, HALF], F32, name=f"pbc_{h}", tag="pbc")
                pbc.append(p)
                for jl in range(4):
                    j = 4 * h + jl
                    nc.tensor.matmul(
                        p[:, jl * 128:(jl + 1) * 128],
                        lhsT=auxh[0:96, B96_OFF + KV * j:B96_OFF + KV * (j + 1)],
                        rhs=h3T[:],
                        start=True, stop=True,
                    )
                nc.vector.tensor_scalar(
                    out=onehot[:, h * HALF:(h + 1) * HALF], in0=p[:],
                    scalar1=iota[:, 0:1], scalar2=None, op0=ALU.is_equal,
                )

            # ---- per half: gather factor rows, multiply blocks, emit output
            for h in range(2):
                pv = pvpool.tile([MV, HALF], F32, name=f"pv_{h}", tag="pv")
                nc.tensor.matmul(
                    pv[:], lhsT=ublk,
                    rhs=onehot[:, h * HALF:(h + 1) * HALF],
                    start=True, stop=True,
                )
                # both-SBUF operands must share a base partition, so read
                # blocks 1/2 straight from PSUM (one PSUM operand per op)
                s0 = wpool.tile([RANK, HALF], F16, name=f"s0_{h}", tag="s0")
                nc.scalar.copy(out=s0[:], in_=pv[0:32, :])
                v01 = wpool.tile([RANK, HALF], F16, name=f"v01_{h}", tag="v01")
                nc.vector.tensor_tensor(out=v01[:], in0=s0[:],
                                        in1=pv[32:64, :], op=ALU.mult)
                vth = wpool.tile([RANK, HALF], F16, name=f"vth_{h}", tag="vth")
                nc.vector.tensor_tensor(out=vth[:], in0=v01[:],
                                        in1=pv[64:96, :], op=ALU.mult)

                # output groups: psum pair (j0, j0+1) so partition p holds
                # consecutive HBM rows 8p+j0, 8p+j0+1 (2 KB contiguous)
                for g in range(2):
                    j0 = 4 * h + 2 * g
                    po2 = popool.tile([128, 2 * EMB], F32, name=f"po_{h}{g}",
                                      tag="po")
                    for jj in range(2):
                        nc.tensor.matmul(
                            po2[:, jj * EMB:(jj + 1) * EMB],
                            lhsT=vth[:, (2 * g + jj) * 128:(2 * g + jj + 1) * 128],
                            rhs=wt[:],
                            start=True, stop=True,
                        )
                    osb = wpool.tile([128, 2 * EMB], F16, name=f"osb_{h}{g}",
                                     tag="osb")
                    if g == 0:
                        nc.scalar.copy(out=osb[:], in_=po2[:])
                    else:
                        nc.vector.tensor_copy(out=osb[:], in_=po2[:])
                    nc.sync.dma_start(
                        out=outv[:, j0:j0 + 2, :],
                        in_=osb[:].rearrange("p (j e) -> p j e", j=2),
                    )

    nc.compile()
    return nc


_CACHE: dict = {}


def _get_nc():
    if "nc" not in _CACHE:
        _CACHE["nc"] = build()
    return _CACHE["nc"]


def run(inputs, **spmd_kwargs):
    nc = _get_nc()
    x = np.ascontiguousarray(inputs["x"].reshape(-1), dtype=np.int32)
    us = [
        np.ascontiguousarray(inputs[f"U{j}"], dtype=np.float32) for j in range(6)
    ]
    auxh = _auxh_table(us)
    in_maps = []
    for i in range(N_CORES):
        in_maps.append({"x": x[i * PER_CORE:(i + 1) * PER_CORE], "auxh": auxh})
    res = run_bass_kernel_spmd(
        nc, in_maps, core_ids=list(range(N_CORES)), **spmd_kwargs
    )
    shards = [np.asarray(res.results[i]["out"]) for i in range(N_CORES)]
    full = np.concatenate(shards, axis=0).reshape(4, 2048, EMB)
    return full.astype(np.float32), res


def kernel(**inputs) -> np.ndarray:
    return run(inputs)[0]


# revision 9
# speedup vs baseline: 1.1907x; 1.0074x over previous
"""CP-factorized embedding lookup on 8 TRN2 NeuronCores (v3).

Reference computes full[a,b,c,d,e,f] = sum_r U0[a,r]*...*U5[f,r], reshapes to a
(50000, 512) table, and gathers rows by x. We never materialize the table:

  out[n, e] = sum_r (U0[a_n,r]*U1[b_n,r]*U2[c_n,r]) * (U3[d,r]*U4[e2,r]*U5[f,r])
            = sum_r V[n, r] * W[e, r]

with v = 1000a + 25b + c and e = 64d + 8e2 + f.

Per core (1024 indices):
  1. x lands once as [8, 128] (4 KB). Digits via exact float-floor tricks,
     split across Scalar-ACT (q=floor(v/25), 25q) and DVE tensor_scalar
     (a=floor(v/1000), 40a, m3=1000*min(v,1)); three DVE tensor_tensor ops
     write h0=a-m3, h1=q-40a, h2=v-25q into a zeroed [96, 128] fp16 tile at
     partition bases 0/32/64 (engine APs must start at multiples of 32).
     Padding folds in: v==0 gives h0=0 which misses every block-0 target
     (iota0 = p-1000), so V0=0 and the output row is zero.
  2. Eight PE matmuls with 0/1 selector weights B96_j broadcast the h rows
     across the 115 stacked vocab-factor partitions; one is_equal
     tensor_scalar per 512-half against the per-partition local index gives
     the fp16 one-hot.
  3. One PE matmul per half gathers all three factor rows at once (block-diag
     [U0;U1;U2] stationary) -> psum[96, 512]; V = product of the three 32-row
     blocks (one scalar copy + two DVE multiplies reading psum).
  4. out chunk: matmul(lhsT=V[:, 128-slice], rhs=W[32, 512]) -> psum, two
     chunks per [128, 1024] psum pair; psum->SBUF fp16 copies split between
     Scalar and Vector; one DMA per 256 output rows.

Schedule notes (all from the HW profile): the B96 selector table (176 KB of
the aux input) is DMA'd as two column-halves on separate rings in parallel
with the small table and x (a single [115, 2 KB] HWDGE DMA only engages ~5
SDMA engines at ~90 GB/s); 28 junk matmuls on memset tiles run first on the
otherwise-idle PE to trip the HAM clock gate from 1.2 to 2.4 GHz before the
real matmuls; W-build and small casts run on the otherwise-idle GpSimd; the
four output DMAs alternate sync/gpsimd queues. Output is fp16 (1 MB instead
of 2 MB of HBM writes; ~2^-11 rounding vs the 2e-2 tolerance), cast to f32
on host. Host-side work is dtype casting and zero-padded packing only.
"""

import numpy as np

import concourse.bass as bass
import concourse.mybir as mybir
import concourse.tile as tile
from concourse import bacc
from concourse.bass_utils import run_bass_kernel_spmd

F32 = mybir.dt.float32
F16 = mybir.dt.float16
I32 = mybir.dt.int32
I16 = mybir.dt.int16
ALU = mybir.AluOpType
ACT = mybir.ActivationFunctionType

N_CORES = 8
PER_CORE = 1024           # indices per core (8192 / 8)
HALF = 512
EMB = 512
RANK = 32
KV = 115                  # 50 + 40 + 25 stacked vocab-factor rows
MV = 96                   # 3 * RANK stacked gather outputs
NJ = 8                    # index chunks of 128 (x laid out [8, 128])
NWARM = 28                # PE warm-up matmuls (HAM clock-gate release)

R25 = float(np.float32(1.0 / 25.0))
R1000 = float(np.float32(1.0 / 1000.0))

# auxh (fp16) layout: [115, AUXH_W]; small tables first so one small DMA
# covers them, the 8 B96_j selectors after (split into two DMAs).
UBLK_OFF = 0     # rows 0:115, cols 0:96      block-diag [U0;U1;U2]
U345_OFF = 96    # rows 0:24,  cols 96:128    stacked U3;U4;U5
ID24_OFF = 128   # rows 0:24,  cols 128:152   identity 24
IOTA_OFF = 152   # rows 0:115, col 152        per-partition compare target
SMALL_W = 154    # pad to even
B96_OFF = 154    # rows 0:96,  cols 154:1074  8x B96_j [96, 115]
AUXH_W = 1074


def _auxh_table(us: list[np.ndarray]) -> np.ndarray:
    aux = np.zeros((KV, AUXH_W), np.float16)
    aux[0:50, UBLK_OFF:UBLK_OFF + 32] = us[0].astype(np.float16)
    aux[50:90, UBLK_OFF + 32:UBLK_OFF + 64] = us[1].astype(np.float16)
    aux[90:115, UBLK_OFF + 64:UBLK_OFF + 96] = us[2].astype(np.float16)
    aux[0:8, U345_OFF:U345_OFF + 32] = us[3].astype(np.float16)
    aux[8:16, U345_OFF:U345_OFF + 32] = us[4].astype(np.float16)
    aux[16:24, U345_OFF:U345_OFF + 32] = us[5].astype(np.float16)
    aux[0:24, ID24_OFF:ID24_OFF + 24] = np.eye(24, dtype=np.float16)
    # block-0 compares h0 = a - m3 = a - 1000 (v>=1), so target is p - 1000;
    # v==0 gives h0 = 0 which misses all of [-1000, -951] -> zero row
    iota = np.concatenate([np.arange(50) - 1000, np.arange(40), np.arange(25)])
    aux[:, IOTA_OFF] = iota.astype(np.float16)
    # B96_j[k, p] = 1 iff k == 32*block(p) + j (h rows at bases 0/32/64)
    blk = np.zeros(KV, np.int64)
    blk[50:90] = 1
    blk[90:115] = 2
    for j in range(NJ):
        m = np.zeros((96, KV), np.float16)
        m[32 * blk + j, np.arange(KV)] = 1.0
        aux[0:96, B96_OFF + KV * j:B96_OFF + KV * (j + 1)] = m
    return aux


def build():
    nc = bacc.Bacc("TRN2", target_bir_lowering=False, debug=False)

    x = nc.dram_tensor("x", [PER_CORE], I32, kind="ExternalInput")
    auxh_d = nc.dram_tensor("auxh", [KV, AUXH_W], F16, kind="ExternalInput")
    out = nc.dram_tensor("out", [PER_CORE, EMB], F16, kind="ExternalOutput")
    outv = out[:].rearrange("(j p) e -> p j e", p=128)  # partition p, row 128j+p

    B96_MID = B96_OFF + KV * 4

    with tile.TileContext(nc) as tc:
        with (
            tc.tile_pool(name="const", bufs=1) as cpool,
            tc.tile_pool(name="work", bufs=2) as wpool,
            tc.tile_pool(name="pbc", bufs=2, space="PSUM") as bcpool,
            tc.tile_pool(name="pv", bufs=2, space="PSUM") as pvpool,
            tc.tile_pool(name="po", bufs=2, space="PSUM") as popool,
        ):
            auxh = cpool.tile([KV, AUXH_W], F16)

            # ---- input DMAs: x + two B96 halves + small tables, on three
            # queues so no single ~90 GB/s HWDGE stream is the bottleneck.
            xt = cpool.tile([NJ, 128], I32)
            nc.sync.dma_start(out=xt[:], in_=x[:].rearrange("(j n) -> j n", j=NJ))
            nc.sync.dma_start(out=auxh[0:96, B96_OFF:B96_MID],
                              in_=auxh_d[0:96, B96_OFF:B96_MID])
            nc.scalar.dma_start(out=auxh[:, 0:SMALL_W],
                                in_=auxh_d[:, 0:SMALL_W])

            # ---- GpSimd stream: junk tiles for PE warm-up, h3T zero-fill,
            # second B96 half, iota cast, W-build products.
            junkw = cpool.tile([128, RANK], F16)
            nc.gpsimd.memset(junkw[:], 0.0)
            junki = cpool.tile([128, 128], F16)
            nc.gpsimd.memset(junki[:], 0.0)
            h3T = cpool.tile([MV, 128], F16)
            nc.gpsimd.memset(h3T[:], 0.0)
            nc.gpsimd.dma_start(out=auxh[0:96, B96_MID:AUXH_W],
                                in_=auxh_d[0:96, B96_MID:AUXH_W])

            ublk = auxh[:, UBLK_OFF:UBLK_OFF + MV]
            u345 = auxh[0:24, U345_OFF:U345_OFF + 32]
            id24 = auxh[0:24, ID24_OFF:ID24_OFF + 24]
            iota16 = auxh[:, IOTA_OFF:IOTA_OFF + 1]

            iota = cpool.tile([KV, 1], F32)
            nc.gpsimd.tensor_copy(out=iota[:], in_=iota16)

            # ---- PE warm-up: HAM releases the clock gate only after ~3.4us
            # of sustained activity; burn it on junk matmuls while DMAs land.
            warm = bcpool.tile([RANK, 128], F32, tag="pbc")
            for i in range(NWARM):
                nc.tensor.matmul(warm[:], lhsT=junkw[:], rhs=junki[:],
                                 start=True, stop=True)

            # ---- W[r, e] = U3[d,r]*U4[e2,r]*U5[f,r],  e = 64d + 8e2 + f
            u345t_ps = pvpool.tile([RANK, 24], F16, tag="pv")
            nc.tensor.transpose(u345t_ps[:], u345, id24)
            u345t = cpool.tile([RANK, 24], F16)
            nc.scalar.copy(out=u345t[:], in_=u345t_ps[:])
            t45 = cpool.tile([RANK, 64], F16)
            nc.gpsimd.tensor_tensor(
                out=t45[:].rearrange("r (e f) -> r e f", e=8),
                in0=u345t[:, 8:16].unsqueeze(2).broadcast_to([RANK, 8, 8]),
                in1=u345t[:, 16:24].unsqueeze(1).broadcast_to([RANK, 8, 8]),
                op=ALU.mult,
            )
            wt = cpool.tile([RANK, EMB], F16)
            nc.gpsimd.tensor_tensor(
                out=wt[:].rearrange("r (d ef) -> r d ef", d=8),
                in0=u345t[:, 0:8].unsqueeze(2).broadcast_to([RANK, 8, 64]),
                in1=t45[:].unsqueeze(1).broadcast_to([RANK, 8, 64]),
                op=ALU.mult,
            )

            # ---- digit decomposition, Scalar and Vector in parallel:
            # S: q = floor(v/25), q25 = 25q;  V: a = floor(v/1000), 40a, m3
            q = cpool.tile([NJ, 128], I16)
            nc.scalar.activation(q[:], xt[:], ACT.Copy,
                                 bias=float(np.float32(-12.0 * R25)), scale=R25)
            q25 = cpool.tile([NJ, 128], I32)
            nc.scalar.activation(q25[:], q[:], ACT.Copy, bias=0.0, scale=25.0)
            a = cpool.tile([NJ, 128], I16)
            nc.vector.tensor_scalar(out=a[:], in0=xt[:], scalar1=-499.5,
                                    scalar2=R1000, op0=ALU.add, op1=ALU.mult)
            a40 = cpool.tile([NJ, 128], I16)
            nc.vector.tensor_scalar(out=a40[:], in0=a[:], scalar1=40.0,
                                    scalar2=None, op0=ALU.mult)
            m3 = cpool.tile([NJ, 128], I16)
            nc.vector.tensor_scalar(out=m3[:], in0=xt[:], scalar1=1.0,
                                    scalar2=1000.0, op0=ALU.min, op1=ALU.mult)

            # h3T rows 0:8 = h0 = a - m3, 32:40 = h1 = q - 40a, 64:72 = h2
            nc.vector.tensor_tensor(out=h3T[0:8, :], in0=a[:], in1=m3[:],
                                    op=ALU.subtract)
            nc.vector.tensor_tensor(out=h3T[32:40, :], in0=q[:], in1=a40[:],
                                    op=ALU.subtract)
            nc.vector.tensor_tensor(out=h3T[64:72, :], in0=xt[:], in1=q25[:],
                                    op=ALU.subtract)

            # ---- broadcast h across factor rows + one-hot compare
            onehot = cpool.tile([KV, PER_CORE], F16)
            for h in range(2):
                p = bcpool.tile([KV, HALF], F32, name=f"pbc_{h}", tag="pbc")
                for jl in range(4):
                    j = 4 * h + jl
                    nc.tensor.matmul(
                        p[:, jl * 128:(jl + 1) * 128],
                        lhsT=auxh[0:96, B96_OFF + KV * j:B96_OFF + KV * (j + 1)],
                        rhs=h3T[:],
                        start=True, stop=True,
                    )
                nc.vector.tensor_scalar(
                    out=onehot[:, h * HALF:(h + 1) * HALF], in0=p[:],
                    scalar1=iota[:, 0:1], scalar2=None, op0=ALU.is_equal,
                )

            # ---- per half: gather, 3-way product, output matmuls, store
            for h in range(2):
                pv = pvpool.tile([MV, HALF], F32, name=f"pv_{h}", tag="pv")
                nc.tensor.matmul(
                    pv[:], lhsT=ublk,
                    rhs=onehot[:, h * HALF:(h + 1) * HALF],
                    start=True, stop=True,
                )
                s0 = wpool.tile([RANK, HALF], F16, name=f"s0_{h}", tag="s0")
                nc.scalar.copy(out=s0[:], in_=pv[0:32, :])
                v01 = wpool.tile([RANK, HALF], F16, name=f"v01_{h}", tag="v01")
                nc.vector.tensor_tensor(out=v01[:], in0=s0[:],
                                        in1=pv[32:64, :], op=ALU.mult)
                vth = wpool.tile([RANK, HALF], F16, name=f"vth_{h}", tag="vth")
                nc.vector.tensor_tensor(out=vth[:], in0=v01[:],
                                        in1=pv[64:96, :], op=ALU.mult)

                for g in range(2):
                    j0 = 4 * h + 2 * g
                    po2 = popool.tile([128, 2 * EMB], F32, name=f"po_{h}{g}",
                                      tag="po")
                    for jj in range(2):
                        nc.tensor.matmul(
                            po2[:, jj * EMB:(jj + 1) * EMB],
                            lhsT=vth[:, (2 * g + jj) * 128:(2 * g + jj + 1) * 128],
                            rhs=wt[:],
                            start=True, stop=True,
                        )
                    osb = wpool.tile([128, 2 * EMB], F16, name=f"osb_{h}{g}",
                                     tag="osb")
                    # split the psum evacuation: Scalar left half, and for
                    # the last group Vector right half (min tail latency)
                    nc.scalar.copy(out=osb[:, 0:EMB], in_=po2[:, 0:EMB])
                    if h == 1 and g == 1:
                        nc.vector.tensor_copy(out=osb[:, EMB:2 * EMB],
                                              in_=po2[:, EMB:2 * EMB])
                    else:
                        nc.scalar.copy(out=osb[:, EMB:2 * EMB],
                                       in_=po2[:, EMB:2 * EMB])
                    eng = nc.sync if g == h else nc.gpsimd
                    eng.dma_start(
                        out=outv[:, j0:j0 + 2, :],
                        in_=osb[:].rearrange("p (j e) -> p j e", j=2),
                    )

    nc.compile()
    return nc


_CACHE: dict = {}


def _get_nc():
    if "nc" not in _CACHE:
        _CACHE["nc"] = build()
    return _CACHE["nc"]


def run(inputs, **spmd_kwargs):
    nc = _get_nc()
    x = np.ascontiguousarray(inputs["x"].reshape(-1), dtype=np.int32)
    us = [
        np.ascontiguousarray(inputs[f"U{j}"], dtype=np.float32) for j in range(6)
    ]
    auxh = _auxh_table(us)
    in_maps = []
    for i in range(N_CORES):
        in_maps.append({"x": x[i * PER_CORE:(i + 1) * PER_CORE], "auxh": auxh})
    res = run_bass_kernel_spmd(
        nc, in_maps, core_ids=list(range(N_CORES)), **spmd_kwargs
    )
    shards = [np.asarray(res.results[i]["out"]) for i in range(N_CORES)]
    full = np.concatenate(shards, axis=0).reshape(4, 2048, EMB)
    return full.astype(np.float32), res


def kernel(**inputs) -> np.ndarray:
    return run(inputs)[0]
